# revision 1
# baseline (speedup 1.0000x reference)
"""Trainium2 Bass kernel for nn_DenseAtt: att[i,j] = sigmoid(x[i]@w1 + x[j]@w2 + b).

Sharding: rows of the (N, N) output are split evenly across 8 NeuronCores
(1250 rows each).  Each core:
  1. loads only its own 1250-row slab of x^T (f-major),
  2. computes s1 = x_slab@w1 + b (per-partition bias layout) and its 1250
     elements of s2 = x@w2 as a single SBUF row,
  3. AllGathers the s2 slabs (10000 floats = 40KB) across the 8 cores,
  4. replicates the full s2 row across 128 partitions with a K=1 ones
     matmul on the (otherwise idle) PE,
  5. streams sigmoid(s2[j] + s1[i]) row-tiles to DRAM through the scalar
     (ACT) engine, per-partition bias = s1.
Memory-bound on the 400MB output write; each core writes its own 50MB slab
and reads only ~1.3MB, so the DMA engines are ~pure output-write.
"""

import math

import numpy as np

import concourse.bacc as bacc
import concourse.tile as tile
from concourse import mybir
from concourse.bass_utils import run_bass_kernel_spmd

N = 10000
F = 256
NCORES = 8
RPC = N // NCORES  # rows per core = 1250
P = 128
CJ = 512  # free-dim chunk (one PSUM bank of f32)
XCJ = 2500  # max column group for the overlapped leading row-tiles
OV_GROUPS = [1250, 1250, 2500, 2500, 2500]  # col groups (first 3 = phase A)
OV_TILES = 2  # leading row-tiles produced chunk-wise behind s2 replication

F32 = mybir.dt.float32

RDIM = {
    "all": 1,
    "main": 2,
    "coll": 3,
    "dma": 4,
    "bigdma": 5,
    "act": 6,
    "dma2": 7,
    "bigdma2": 8,
    "dma3": 9,
}


def build_bass(reps=1, timing=False, rep_scope="all"):
    """Per-core SPMD program.  Inputs (per core):
    xts (F, RPC) : x^T slab of this core's rows (f-major)
    wc  (F, 2)   : [w1 | w2] as columns
    bb  (P, 1)   : bias replicated per partition
    out (RPC, N) : this core's output slab

    reps/timing/rep_scope: differential-timing variants (see test.py).
    """
    nc = bacc.Bacc("TRN2", target_bir_lowering=False, debug=False, num_devices=NCORES)
    xts = nc.declare_dram_parameter("xts", [F, RPC], F32, isOutput=False)
    wc = nc.declare_dram_parameter("wc", [F, 2], F32, isOutput=False)
    bb = nc.declare_dram_parameter("bb", [P, 1], F32, isOutput=False)
    rtag = None
    if reps > 1 or timing:
        # dummy input whose shape encodes (reps, rep_scope): the neuron
        # compile cache can collide variants otherwise
        rdim = RDIM[rep_scope]
        rtag = nc.declare_dram_parameter("rtag", [rdim, reps], F32, isOutput=False)
    if timing:
        # timing mode: full-size writes go to internal DRAM so the (noisy,
        # ~40ms) 400MB axon output path is replaced by a tiny output
        out = nc.dram_tensor("out_scratch", [RPC, N], F32)
        ok = nc.declare_dram_parameter("ok", [1, 4], F32, isOutput=True)
    else:
        out = nc.declare_dram_parameter("out", [RPC, N], F32, isOutput=True)
        ok = None

    nrt = math.ceil(RPC / P)  # row tiles per core (9x128 + 98)

    with tile.TileContext(nc) as tc:
        with (
            tc.tile_pool(name="consts", bufs=1) as consts,
            tc.tile_pool(name="s2", bufs=1) as s2pool,
            tc.tile_pool(name="xsp", bufs=1) as xsp,
            tc.tile_pool(name="psum", bufs=3, space="PSUM") as psum,
            tc.tile_pool(name="psum1", bufs=2, space="PSUM") as psum1,
            tc.tile_pool(name="psum2", bufs=3, space="PSUM") as psum2,
            tc.tile_pool(name="oovp", bufs=5) as oovp,
            tc.tile_pool(name="outp", bufs=2) as outp,
            tc.tile_pool(name="bigp", bufs=1) as bigp,
            tc.tile_pool(name="s2rbp", bufs=2) as s2rbp,
            tc.tile_pool(name="dram", bufs=1, space="DRAM") as dram,
        ):
          if rtag is not None:
            rtag_sb = consts.tile(list(rtag.shape), F32, tag="rtag")
            nc.scalar.dma_start(out=rtag_sb, in_=rtag[:, :])
          n_outer = reps if rep_scope == "all" else 1
          n_main = reps if rep_scope == "main" else 1
          for _rep in range(n_outer):
            # --- constants ---
            wc_sb = consts.tile([P, 2, 2], F32)
            nc.scalar.dma_start(out=wc_sb[:, 0, :], in_=wc[0:P, :])
            nc.scalar.dma_start(out=wc_sb[:, 1, :], in_=wc[P : 2 * P, :])
            b_sb = consts.tile([P, 1], F32)
            nc.scalar.dma_start(out=b_sb, in_=bb[:, :])
            ones_sb = consts.tile([1, P], F32)
            nc.vector.memset(ones_sb, 1.0)

            # --- own slab of x^T: one resident tile, 2 DMAs (1.25MB) ---
            xts_sb = xsp.tile([P, 2, RPC], F32)
            for sj in range(0, RPC, CJ):
                cw = min(CJ, RPC - sj)
                nc.sync.dma_start(
                    out=xts_sb[:, 0, sj : sj + cw], in_=xts[0:P, sj : sj + cw]
                )
                nc.sync.dma_start(
                    out=xts_sb[:, 1, sj : sj + cw],
                    in_=xts[P : 2 * P, sj : sj + cw],
                )

            # --- own 1250 elements of s2 = x @ w2, as a single row (first,
            # so the AllGather launches as early as possible) ---
            s2s_sb = consts.tile([1, RPC], F32)
            for sj in range(0, RPC, CJ):
                cw = min(CJ, RPC - sj)
                pss = psum2.tile([1, CJ], F32, tag="pss")
                nc.tensor.matmul(
                    out=pss[0:1, :cw],
                    lhsT=wc_sb[:, 0, 1:2],
                    rhs=xts_sb[:, 0, sj : sj + cw],
                    start=True,
                    stop=False,
                )
                nc.tensor.matmul(
                    out=pss[0:1, :cw],
                    lhsT=wc_sb[:, 1, 1:2],
                    rhs=xts_sb[:, 1, sj : sj + cw],
                    start=False,
                    stop=True,
                )
                nc.vector.tensor_copy(
                    out=s2s_sb[0:1, sj : sj + cw], in_=pss[0:1, :cw]
                )

            # --- AllGather the s2 slabs: 5KB in, 40KB out; s1 overlaps it ---
            if rep_scope == "coll":
                # marginal cost of one collective round: stage + AllGather +
                # small readback (serialized across reps by the shared DRAM
                # buffers)
                for _r in range(reps - 1):
                    in_r = dram.tile([1, RPC], F32, tag="in_b", name="in_r")
                    out_r = dram.tile([1, N], F32, tag="out_b", name="out_r")
                    nc.scalar.dma_start(out=in_r[:, :], in_=s2s_sb[:, :])
                    nc.gpsimd.collective_compute(
                        "AllGather",
                        mybir.AluOpType.bypass,
                        replica_groups=[list(range(NCORES))],
                        ins=[in_r[:, :]],
                        outs=[out_r[:, :]],
                    )
                    s2rb = s2rbp.tile([1, RPC], F32, tag="s2rb")
                    nc.scalar.dma_start(out=s2rb[:, :], in_=out_r[0:1, 0:RPC])
            in_b = dram.tile([1, RPC], F32, tag="in_b")
            out_b = dram.tile([1, N], F32, tag="out_b")
            nc.scalar.dma_start(out=in_b[:, :], in_=s2s_sb[:, :])
            nc.gpsimd.collective_compute(
                "AllGather",
                mybir.AluOpType.bypass,
                replica_groups=[list(range(NCORES))],
                ins=[in_b[:, :]],
                outs=[out_b[:, :]],
            )

            # --- s1 = x_slab @ w1 + b (runs during the collective) ---
            s1_sb = consts.tile([P, nrt], F32)
            for t in range(nrt):
                r0 = t * P
                rt = min(P, RPC - r0)
                ps1 = psum1.tile([P, 8], F32, tag="ps1")
                nc.tensor.matmul(
                    out=ps1[:rt, 0:1],
                    lhsT=xts_sb[:, 0, r0 : r0 + rt],
                    rhs=wc_sb[:, 0, 0:1],
                    start=True,
                    stop=False,
                )
                nc.tensor.matmul(
                    out=ps1[:rt, 0:1],
                    lhsT=xts_sb[:, 1, r0 : r0 + rt],
                    rhs=wc_sb[:, 1, 0:1],
                    start=False,
                    stop=True,
                )
                nc.vector.tensor_scalar_add(
                    out=s1_sb[:rt, t : t + 1], in0=ps1[:rt, 0:1], scalar1=b_sb[:rt, :]
                )

            s2row_sb = consts.tile([1, N], F32)
            nc.scalar.dma_start(out=s2row_sb[:, :], in_=out_b[:, :])

            if rep_scope in ("bigdma", "bigdma2"):
                # replicate ALL columns, prepare two full-row tiles, then
                # stream 5MB DMAs only
                s2_rep = s2pool.tile([P, N], F32)
                for sj in range(0, N, CJ):
                    cw = min(CJ, N - sj)
                    ps = psum.tile([P, CJ], F32, tag="ps")
                    nc.tensor.matmul(
                        out=ps[:, :cw],
                        lhsT=ones_sb,
                        rhs=s2row_sb[0:1, sj : sj + cw],
                        start=True,
                        stop=True,
                    )
                    nc.vector.tensor_copy(
                        out=s2_rep[:, sj : sj + cw], in_=ps[:, :cw]
                    )
                big0 = bigp.tile([P, N], F32, tag="big0")
                big1 = bigp.tile([P, N], F32, tag="big1")
                for big, t in ((big0, 0), (big1, 1)):
                    nc.scalar.activation(
                        out=big[:, :],
                        in_=s2_rep[:, :],
                        func=mybir.ActivationFunctionType.Sigmoid,
                        bias=s1_sb[:, t : t + 1],
                        scale=1.0,
                    )
                for _r in range(reps):
                    for t in range(5):
                        r0 = t * P
                        eng = (
                            nc.sync
                            if (rep_scope == "bigdma" or t % 2 == 0)
                            else nc.scalar
                        )
                        eng.dma_start(
                            out=out[r0 : r0 + P, :],
                            in_=(big0 if t % 2 == 0 else big1)[:, :],
                        )
            else:
                # --- replicate s2 across partitions (K=1 ones-matmul on PE) ---
                # Phase A: first-half columns produced column-major right behind
                # the replication stream (all row-tiles per group), so output DMA
                # saturates immediately after the collective.
                s2_rep = s2pool.tile([P, N], F32)
                jc = 0
                for xw in OV_GROUPS[:3]:
                    for sj in range(0, xw, CJ):
                        cw = min(CJ, xw - sj)
                        ps = psum.tile([P, CJ], F32, tag="ps")
                        nc.tensor.matmul(
                            out=ps[:, :cw],
                            lhsT=ones_sb,
                            rhs=s2row_sb[0:1, jc + sj : jc + sj + cw],
                            start=True,
                            stop=True,
                        )
                        nc.vector.tensor_copy(
                            out=s2_rep[:, jc + sj : jc + sj + cw], in_=ps[:, :cw]
                        )
                    for t in range(nrt):
                        r0 = t * P
                        rt = min(P, RPC - r0)
                        o_ov = oovp.tile([P, XCJ], F32, tag="o_ov", name=f"oov{t}_{jc}")
                        nc.scalar.activation(
                            out=o_ov[:rt, :xw],
                            in_=s2_rep[:rt, jc : jc + xw],
                            func=mybir.ActivationFunctionType.Sigmoid,
                            bias=s1_sb[:rt, t : t + 1],
                            scale=1.0,
                        )
                        nc.sync.dma_start(
                            out=out[r0 : r0 + rt, jc : jc + xw],
                            in_=o_ov[:rt, :xw],
                        )
                    jc += xw
                # Phase B: replicate the remaining columns (runs under phase A's
                # write stream)
                for xw in OV_GROUPS[3:]:
                    for sj in range(0, xw, CJ):
                        cw = min(CJ, xw - sj)
                        ps = psum.tile([P, CJ], F32, tag="ps")
                        nc.tensor.matmul(
                            out=ps[:, :cw],
                            lhsT=ones_sb,
                            rhs=s2row_sb[0:1, jc + sj : jc + sj + cw],
                            start=True,
                            stop=True,
                        )
                        nc.vector.tensor_copy(
                            out=s2_rep[:, jc + sj : jc + sj + cw], in_=ps[:, :cw]
                        )
                    jc += xw

                # Phase C: second-half columns, one efficient 2.56MB piece per
                # row-tile
                if rep_scope in ("dma", "dma2", "dma3"):
                    d0 = outp.tile([P, N // 2], F32, tag="o_t", name="d0")
                    d1 = outp.tile([P, N // 2], F32, tag="o_t", name="d1")
                    for d, t in ((d0, 0), (d1, 1)):
                        nc.scalar.activation(
                            out=d[:, :],
                            in_=s2_rep[:, N // 2 : N],
                            func=mybir.ActivationFunctionType.Sigmoid,
                            bias=s1_sb[:, t : t + 1],
                            scale=1.0,
                        )
                    for _r in range(reps):
                        for t in range(nrt):
                            r0 = t * P
                            rt = min(P, RPC - r0)
                            if rep_scope == "dma":
                                eng = nc.sync
                            elif rep_scope == "dma2":
                                eng = nc.sync if t % 2 == 0 else nc.scalar
                            else:
                                eng = (nc.sync, nc.scalar, nc.gpsimd)[t % 3]
                            eng.dma_start(
                                out=out[r0 : r0 + rt, N // 2 : N],
                                in_=(d0 if t % 2 == 0 else d1)[:rt, :],
                            )
                elif rep_scope == "act":
                    o_t = None
                    for _r in range(reps):
                        for t in range(nrt):
                            rt = min(P, RPC - t * P)
                            o_t = outp.tile([P, N // 2], F32, tag="o_t")
                            nc.scalar.activation(
                                out=o_t[:rt, :],
                                in_=s2_rep[:rt, N // 2 : N],
                                func=mybir.ActivationFunctionType.Sigmoid,
                                bias=s1_sb[:rt, t : t + 1],
                                scale=1.0,
                            )
                    nc.sync.dma_start(out=out[0:P, N // 2 : N], in_=o_t[:, :])
                else:
                    for _mrep in range(n_main):
                      for t in range(nrt):
                        r0 = t * P
                        rt = min(P, RPC - r0)
                        o_t = outp.tile([P, N // 2], F32, tag="o_t")
                        nc.scalar.activation(
                            out=o_t[:rt, :],
                            in_=s2_rep[:rt, N // 2 : N],
                            func=mybir.ActivationFunctionType.Sigmoid,
                            bias=s1_sb[:rt, t : t + 1],
                            scale=1.0,
                        )
                        nc.sync.dma_start(
                            out=out[r0 : r0 + rt, N // 2 : N],
                            in_=o_t[:rt, :],
                        )
          if ok is not None:
            # read back from the scratch output so walrus can't dead-store-
            # eliminate the full-size writes (memloc now has a reader)
            okt = consts.tile([1, 4], F32, tag="okt")
            nc.sync.dma_start(out=okt, in_=out[0:1, 0:4])
            nc.sync.dma_start(out=ok[:, :], in_=okt)
    nc.compile()
    return nc


_NC = {}


def _get_nc(reps=1, timing=False, rep_scope="all"):
    key = (reps, timing, rep_scope)
    if key not in _NC:
        _NC[key] = build_bass(reps=reps, timing=timing, rep_scope=rep_scope)
    return _NC[key]


def make_in_maps(x, w, b):
    xT = np.ascontiguousarray(x.T)  # (F, N)
    wc = np.ascontiguousarray(np.stack([w[0, :F], w[0, F:]], axis=1))  # (F, 2)
    bb = np.full((P, 1), np.float32(b[0]), dtype=np.float32)
    in_maps = []
    for c in range(NCORES):
        xts = np.ascontiguousarray(xT[:, c * RPC : (c + 1) * RPC])
        in_maps.append({"xts": xts, "wc": wc, "bb": bb})
    return in_maps


def kernel(x, adj, w, b):
    x = np.asarray(x, dtype=np.float32)
    w = np.asarray(w, dtype=np.float32)
    b = np.asarray(b, dtype=np.float32)
    nc = _get_nc()
    in_maps = make_in_maps(x, w, b)
    res = run_bass_kernel_spmd(nc, in_maps, list(range(NCORES)))
    return np.concatenate([res.results[c]["out"] for c in range(NCORES)], axis=0)

